# revision 14
# baseline (speedup 1.0000x reference)
"""MoE routing kernel (nn_DecFCSwitch) for 8 Trainium2 NeuronCores.

Reference computes all 16 expert branches for every token and then
selects one per token.  Only the selected branch matters, so:

  host:   sort tokens by expert, pad each expert's tokens to capacity C,
          relu(x), cast to fp8e4m3 (weights pre-scaled by 256 so all
          values sit in e4m3's normal range), lay every tensor out
          partition-major so each DMA is one big contiguous run per
          partition (full 360 B/ns bus, no <512B penalty).  ALL biases
          fold into the host-side gather: b' = W_out @ b_in + b_out is
          exact in f32 and frees the device from bias handling.
  device: expert-parallel SPMD - core i owns experts {2i, 2i+1}.
          Both layers run as fp8 DoubleRow matmuls (256-deep
          contraction, 0.5 cycles/row).  PSUM evictions are pure
          scale(1/256)+fp8-cast, grouped two PSUM banks per instruction
          (bias-free, so one instruction can span banks), split between
          ACT and DVE.  Stores ride HWDGE, issued by the engine that
          finished the chunk's last eviction (no cross-engine hop).
          Warm-up matmuls piggyback on the next real PSUM pair to hold
          the PE p-state without costing a PSUM bank.
  host:   decode fp8 -> f32, scatter rows to token order,
          out = x + sel + b'.
"""

import os
import sys

import numpy as np

for _p in ("/opt/trn_rl_repo", "/root/.axon_site/_ro/trn_rl_repo"):
    if os.path.isdir(_p) and _p not in sys.path:
        sys.path.insert(0, _p)

import ml_dtypes

B, D, S, NB = 4096, 1024, 256, 16
NCORES = 8
EPC = NB // NCORES  # experts per core
KD = D // 128  # d-dim 128-tiles
KS = S // 128  # s-dim 128-tiles
K2 = D // 256  # d-dim 256-tiles (DoubleRow contraction chunks)

F8 = ml_dtypes.float8_e4m3
WSCALE = 256.0  # host-side weight scale (power of two; undone at eviction)

_programs = {}  # C -> compiled Bacc program
LAST_RESULT = None  # BassKernelResults of the most recent run (for test.py)

# Warm-matmul counts injected before each real PSUM pair's matmul group,
# in allocation order: (L1e0, e0m01, e0m23, e0m45, e0m67, L1e1, e1m01,
# e1m23, e1m45, e1m67) - zero-padded.
WARM = (8, 2, 0, 0, 0, 0, 0, 0, 0, 0)
# Eviction engine per job, 0=ACT 1=DVE, jobs in program order:
# (L1e0-t0, L1e0-t1, e0m01, e0m23, e0m45, e0m67,
#  L1e1-t0, L1e1-t1, e1m01, e1m23, e1m45, e1-m6, e1-m7)
EVICT = (0, 1, 0, 1, 0, 1, 0, 1, 0, 1, 0, 1, 0)


def _build_program(C, warm=WARM, ev=EVICT):
    import concourse.mybir as mybir
    import concourse.tile as tile
    from concourse import bacc

    f8 = mybir.dt.float8e4
    bf16 = mybir.dt.bfloat16
    f32 = mybir.dt.float32
    copy_f = mybir.ActivationFunctionType.Copy
    DR = mybir.MatmulPerfMode.DoubleRow

    nc = bacc.Bacc()
    # All DRAM params partition-major: [128, cols], cols contiguous per row.
    hT = nc.declare_dram_parameter("hT", [128, KD * EPC * C], f8, isOutput=False)
    w1 = nc.declare_dram_parameter("w1", [128, EPC * 2048], f8, isOutput=False)
    w2 = nc.declare_dram_parameter("w2", [128, EPC * 2048], f8, isOutput=False)
    yT = nc.declare_dram_parameter("yT", [128, EPC * KD * C], f8, isOutput=True)

    HC = KD * EPC * C // 2  # h half-load cols
    warm = list(warm) + [0] * 16
    wslot = iter(warm)

    with tile.TileContext(nc) as tc:
        with (
            tc.tile_pool(name="h", bufs=1) as h_pool,
            tc.tile_pool(name="w1p", bufs=1) as w1_pool,
            tc.tile_pool(name="w2p", bufs=1) as w2_pool,
            tc.tile_pool(name="hid", bufs=2) as hid_pool,
            tc.tile_pool(name="yout", bufs=2) as y_pool,
            tc.tile_pool(name="warm", bufs=1) as warm_pool,
            tc.tile_pool(name="ps1", bufs=1, space="PSUM") as ps1_pool,
            tc.tile_pool(name="ps2", bufs=3, space="PSUM") as ps2_pool,
        ):
            # Warm operand for p-state-holding dummy matmuls.
            wz = warm_pool.tile([128, 512], bf16, tag="wz")

            # --- SBUF tiles -------------------------------------------------
            ht = h_pool.tile([128, KD * EPC * C], f8, tag="h")
            w1t = w1_pool.tile([128, EPC * 2048], f8, tag="w1")
            w2t = w2_pool.tile([128, EPC * 2048], f8, tag="w2")
            hid = [
                hid_pool.tile([128, KS * C], f8, tag=f"hid{e}", name=f"hid{e}")
                for e in range(EPC)
            ]
            ybig = [
                y_pool.tile([128, KD * C], f8, tag=f"y{e}", name=f"y{e}")
                for e in range(EPC)
            ]

            # --- loads, in first-use order ----------------------------------
            # First load rides SWDGE (Pool): its descriptor gen starts at t~60
            # vs the HWDGE path's ~690, so the DMA engines start sooner.
            nc.gpsimd.dma_start(out=w1t[:, 0:2048], in_=w1[:, 0:2048])  # w1[e0]
            nc.gpsimd.memset(wz[:], 0)

            def load_cols(dst, src, c0, c1):
                nc.sync.dma_start(out=dst[:, c0:c1], in_=src[:, c0:c1])

            load_cols(ht, hT, 0, HC)  # h first half (k 0..3)
            load_cols(ht, hT, HC, 2 * HC)  # h second half (k 4..7)
            load_cols(w2t, w2, 0, 2048)  # w2[e0]
            load_cols(w1t, w1, 2048, 4096)  # w1[e1]
            load_cols(w2t, w2, 2048, 2048 + 1536)  # w2[e1] m0-5
            load_cols(w2t, w2, 2048 + 1536, 4096)  # w2[e1] m6-7

            ht_v = ht[:].rearrange("p (k e c) -> p k e c", k=KD, e=EPC)

            def pair(pool):
                # Two PSUM banks: matmuls target one bank each; a single
                # eviction spans both.  Warm matmuls (if any) precede the
                # real groups on the same banks - same engine, in-order.
                pp = pool.tile([128, 2 * 512], f32, name="pp")
                for _ in range(next(wslot)):
                    nc.tensor.matmul(
                        pp[:, 0:512], lhsT=wz[:, 0:128], rhs=wz[:],
                        start=True, stop=True,
                    )
                return pp

            def l1_matmul(e, t, k2, out_ap):
                base = ((e * KS + t) * K2 + k2) * 256
                nc.tensor.matmul(
                    out_ap,
                    lhsT=w1t[:, base : base + 256].rearrange("p (i m) -> p i m", i=2),
                    rhs=ht_v[:, 2 * k2 : 2 * k2 + 2, e],
                    start=(k2 == 0),
                    stop=(k2 == K2 - 1),
                    perf_mode=DR,
                )

            def l2_matmul(e, m, out_ap, hid_v):
                base = (e * KD + m) * 256
                nc.tensor.matmul(
                    out_ap,
                    lhsT=w2t[:, base : base + 256].rearrange("p (i m) -> p i m", i=2),
                    rhs=hid_v,
                    start=True,
                    stop=True,
                    perf_mode=DR,
                )

            ACT, DVE = 0, 1

            def evict(eng, dst_cols, src):
                # out = ps/WSCALE cast to fp8
                if eng == ACT:
                    nc.scalar.activation(dst_cols, src, copy_f, scale=1.0 / WSCALE)
                else:
                    nc.vector.tensor_scalar_mul(dst_cols, src, 1.0 / WSCALE)

            def evict_pair(eng, dst_cols, pp):
                src = pp[:].rearrange("p (t x) -> p t x", t=2)[:, :, 0:C]
                evict(eng, dst_cols.rearrange("p (t c) -> p t c", t=2), src)

            def store(e, m0, m1, issuer=None):
                # Mid-stream stores ride SP (idle after loads, keeps ACT.SEQ
                # free to dispatch evictions); the final one rides ACT right
                # behind its own last eviction.
                (issuer or nc.sync).dma_start(
                    out=yT[:, (e * KD + m0) * C : (e * KD + m1) * C],
                    in_=ybig[e][:, m0 * C : m1 * C],
                )

            ji = iter(ev)

            for e in range(EPC):
                # --- L1: hid[s,c] over 4 DoubleRow chunks of d --------------
                pp1 = pair(ps1_pool)
                for k2 in range(K2):
                    for t in range(KS):
                        l1_matmul(e, t, k2, pp1[:, t * 512 : t * 512 + C])
                # Split eviction: one single per engine so both start early.
                for t in range(KS):
                    evict(
                        next(ji),
                        hid[e][:, t * C : (t + 1) * C],
                        pp1[:, t * 512 : t * 512 + C],
                    )

                # --- L2: one DoubleRow matmul per 128-row output tile -------
                hid_v = hid[e][:].rearrange("p (i c) -> p i c", i=KS)
                for mp in range(KD // 2):  # pairs m = 2mp, 2mp+1
                    lo = 2 * mp
                    if e == 1 and mp == 3:
                        # Final pair lands in the (now free) L1 bank pair and
                        # evicts as two singles on both engines in parallel.
                        pp = pair(ps1_pool)
                        l2_matmul(e, lo, pp[:, 0:C], hid_v)
                        l2_matmul(e, lo + 1, pp[:, 512 : 512 + C], hid_v)
                        evict(next(ji), ybig[e][:, lo * C : (lo + 1) * C], pp[:, 0:C])
                        evict(
                            next(ji),
                            ybig[e][:, (lo + 1) * C : (lo + 2) * C],
                            pp[:, 512 : 512 + C],
                        )
                    else:
                        pp = pair(ps2_pool)
                        l2_matmul(e, lo, pp[:, 0:C], hid_v)
                        l2_matmul(e, lo + 1, pp[:, 512 : 512 + C], hid_v)
                        evict_pair(
                            next(ji), ybig[e][:, lo * C : (lo + 2) * C], pp
                        )
                    if mp == 1:  # m0-3 ready
                        store(e, 0, 4)
                    elif mp == 2 and e == 1:  # m4-5 ready
                        store(e, 4, 6)
                    elif mp == 3:
                        if e == 0:
                            store(e, 4, 8)
                        else:
                            store(e, 6, 8, issuer=nc.scalar)

    nc.compile()
    return nc


def kernel(x, y_index, W_in, b_in, W_out, b_out):
    global LAST_RESULT
    from concourse.bass_utils import run_bass_kernel_spmd

    x = np.asarray(x, dtype=np.float32)
    W_in = np.asarray(W_in, dtype=np.float32)
    b_in = np.asarray(b_in, dtype=np.float32)
    W_out = np.asarray(W_out, dtype=np.float32)
    b_out = np.asarray(b_out, dtype=np.float32)
    eidx = np.asarray(y_index).reshape(-1).astype(np.int64)

    counts = np.bincount(eidx, minlength=NB)
    C = max(276, int(-(-counts.max() // 4) * 4))  # capacity per expert

    if C > 512:
        # Extreme expert skew would overflow a PSUM bank (512 f32 free dim);
        # fall back to exact host math rather than ship a broken program.
        out = np.empty_like(x)
        h_full = np.maximum(x, 0.0)
        for e in range(NB):
            m = eidx == e
            if m.any():
                hid = h_full[m] @ W_in[e].T + b_in[e]
                out[m] = x[m] + hid @ W_out[e].T + b_out[e]
        return out

    # --- host dispatch: group tokens by expert ---------------------------
    order = np.argsort(eidx, kind="stable")
    starts = np.zeros(NB + 1, dtype=np.int64)
    np.cumsum(counts, out=starts[1:])

    h = np.maximum(x, 0.0)
    Xg = np.zeros((NB, C, D), dtype=np.float32)
    for e in range(NB):
        toks = order[starts[e] : starts[e + 1]]
        Xg[e, : counts[e]] = h[toks]

    # Fold both biases into one host-side per-expert vector (exact f32).
    bML = np.einsum("eds,es->ed", W_out, b_in) + b_out  # [NB, D]

    # hT: [core, 128, (k, e, c)] - value = h[token (e,c), 128k + p]
    hT_all = np.ascontiguousarray(
        Xg.astype(F8)
        .reshape(NCORES, EPC, C, KD, 128)
        .transpose(0, 4, 3, 1, 2)
        .reshape(NCORES, 128, KD * EPC * C)
    )
    # w1: [core, 128, (e, t, k2, i, m)] = W_in[e, 128t+m, 256k2+128i+p] * 256
    w1_all = np.ascontiguousarray(
        (W_in * WSCALE)
        .astype(F8)
        .reshape(NCORES, EPC, KS, 128, K2, 2, 128)
        .transpose(0, 6, 1, 2, 4, 5, 3)
        .reshape(NCORES, 128, EPC * 2048)
    )
    # w2: [core, 128, (e, m, i, j)] = W_out[e, 128m+j, 128i+p] * 256
    w2_all = np.ascontiguousarray(
        (W_out * WSCALE)
        .astype(F8)
        .reshape(NCORES, EPC, KD, 128, KS, 128)
        .transpose(0, 5, 1, 2, 4, 3)
        .reshape(NCORES, 128, EPC * 2048)
    )

    if C not in _programs:
        _programs[C] = _build_program(C)
    nc = _programs[C]

    in_maps = [
        {"hT": hT_all[i], "w1": w1_all[i], "w2": w2_all[i]} for i in range(NCORES)
    ]

    trace = bool(int(os.environ.get("KERNEL_TRACE", "0")))
    res = run_bass_kernel_spmd(nc, in_maps, list(range(NCORES)), trace=trace)
    LAST_RESULT = res

    # --- host gather: decode fp8, add folded bias, scatter ---------------
    out = np.empty_like(x)
    Yg = np.stack(
        [np.asarray(r["yT"]).astype(np.float32) for r in res.results]
    )  # [NCORES, 128, EPC*KD*C]
    Yg = (
        Yg.reshape(NCORES, 128, EPC, KD, C)
        .transpose(0, 2, 4, 3, 1)
        .reshape(NB, C, D)
    )
    for e in range(NB):
        toks = order[starts[e] : starts[e + 1]]
        out[toks] = x[toks] + Yg[e, : counts[e]] + bML[e]
    return out


# revision 22
# speedup vs baseline: 1.1768x; 1.1768x over previous
"""MoE routing kernel (nn_DecFCSwitch) for 8 Trainium2 NeuronCores.

Reference computes all 16 expert branches for every token and then
selects one per token.  Only the selected branch matters, so:

  host:   sort tokens by expert, pad each expert's tokens to capacity C,
          relu(x), cast to fp8e4m3 (weights pre-scaled by 256 so all
          values sit in e4m3's normal range), lay every tensor out
          partition-major so each DMA is one big contiguous run per
          partition (full 360 B/ns bus, no <512B penalty).  ALL biases
          fold into the host-side gather: b' = W_out @ b_in + b_out is
          exact in f32 and frees the device from bias handling.
  device: expert-parallel SPMD - core i owns experts {2i, 2i+1}.
          Both layers run as fp8 DoubleRow matmuls (256-deep
          contraction, 0.5 cycles/row).  PSUM evictions are pure
          scale(1/256)+fp8-cast, grouped two PSUM banks per instruction
          (bias-free, so one instruction can span banks), split between
          ACT and DVE.  Stores ride HWDGE, issued by the engine that
          finished the chunk's last eviction (no cross-engine hop).
          Warm-up matmuls piggyback on the next real PSUM pair to hold
          the PE p-state without costing a PSUM bank.
  host:   decode fp8 -> f32, scatter rows to token order,
          out = x + sel + b'.
"""

import os
import sys

import numpy as np

for _p in ("/opt/trn_rl_repo", "/root/.axon_site/_ro/trn_rl_repo"):
    if os.path.isdir(_p) and _p not in sys.path:
        sys.path.insert(0, _p)

import ml_dtypes

B, D, S, NB = 4096, 1024, 256, 16
NCORES = 8
EPC = NB // NCORES  # experts per core
KD = D // 128  # d-dim 128-tiles
KS = S // 128  # s-dim 128-tiles
K2 = D // 256  # d-dim 256-tiles (DoubleRow contraction chunks)

F8 = ml_dtypes.float8_e4m3
WSCALE = 256.0  # host-side weight scale (power of two; undone at eviction)

_programs = {}  # C -> compiled Bacc program
LAST_RESULT = None  # BassKernelResults of the most recent run (for test.py)

# Warm-matmul counts injected before each real PSUM pair's matmul group,
# in allocation order: (L1e0, e0m01, e0m23, e0m45, e0m67, L1e1, e1m01,
# e1m23, e1m45, e1m67) - zero-padded.
WARM = (5, 0, 0, 0, 0, 0, 0, 0, 0, 0)
# Eviction engine per job, 0=ACT 1=DVE, jobs in program order:
# (L1e0, e0m01, e0m23, e0m45, e0m67, L1e1, e1m01, e1m23, e1m45, e1m67)
# L1 evictions ride one engine as a single pair op: DVE pays a ~535ns
# PSUM-access latency on the first op of a burst, and splitting the
# final pair across engines couples it to the slower engine's queue.
EVICT = (0, 1, 0, 0, 1, 1, 0, 0, 1, 1)
# Load order permutation of (h1, h2, w2e0, w1e1, w2e1a, w2e1b); w1e0 first.
LOADS = ("h1", "h2", "w2e0", "w1e1", "w2e1a", "w2e1b")


def _build_program(C, warm=WARM, ev=EVICT, loads=LOADS):
    import concourse.mybir as mybir
    import concourse.tile as tile
    from concourse import bacc

    f8 = mybir.dt.float8e4
    bf16 = mybir.dt.bfloat16
    f32 = mybir.dt.float32
    copy_f = mybir.ActivationFunctionType.Copy
    DR = mybir.MatmulPerfMode.DoubleRow

    nc = bacc.Bacc()
    # All DRAM params partition-major: [128, cols], cols contiguous per row.
    hT = nc.declare_dram_parameter("hT", [128, KD * EPC * C], f8, isOutput=False)
    w1 = nc.declare_dram_parameter("w1", [128, EPC * 2048], f8, isOutput=False)
    w2 = nc.declare_dram_parameter("w2", [128, EPC * 2048], f8, isOutput=False)
    yT = nc.declare_dram_parameter("yT", [128, EPC * KD * C], f8, isOutput=True)

    HC = KD * EPC * C // 2  # h half-load cols
    warm = list(warm) + [0] * 16
    wslot = iter(warm)

    with tile.TileContext(nc) as tc:
        with (
            tc.tile_pool(name="h", bufs=1) as h_pool,
            tc.tile_pool(name="w1p", bufs=1) as w1_pool,
            tc.tile_pool(name="w2p", bufs=1) as w2_pool,
            tc.tile_pool(name="hid", bufs=2) as hid_pool,
            tc.tile_pool(name="yout", bufs=2) as y_pool,
            tc.tile_pool(name="warm", bufs=1) as warm_pool,
            tc.tile_pool(name="ps1", bufs=1, space="PSUM") as ps1_pool,
            tc.tile_pool(name="ps2", bufs=3, space="PSUM") as ps2_pool,
        ):
            # Warm operand for p-state-holding dummy matmuls.
            wz = warm_pool.tile([128, 512], bf16, tag="wz")

            # --- SBUF tiles -------------------------------------------------
            ht = h_pool.tile([128, KD * EPC * C], f8, tag="h")
            w1t = w1_pool.tile([128, EPC * 2048], f8, tag="w1")
            w2t = w2_pool.tile([128, EPC * 2048], f8, tag="w2")
            hid = [
                hid_pool.tile([128, KS * C], f8, tag=f"hid{e}", name=f"hid{e}")
                for e in range(EPC)
            ]
            ybig = [
                y_pool.tile([128, KD * C], f8, tag=f"y{e}", name=f"y{e}")
                for e in range(EPC)
            ]

            # --- loads (HWDGE on SP), in first-use order --------------------
            nc.gpsimd.memset(wz[:], 0)

            def load_cols(dst, src, c0, c1):
                nc.sync.dma_start(out=dst[:, c0:c1], in_=src[:, c0:c1])

            load_chunks = {
                "h1": (ht, hT, 0, HC),
                "h2": (ht, hT, HC, 2 * HC),
                "w2e0": (w2t, w2, 0, 2048),
                "w1e1": (w1t, w1, 2048, 4096),
                "w2e1a": (w2t, w2, 2048, 2048 + 1536),
                "w2e1b": (w2t, w2, 2048 + 1536, 4096),
            }
            load_cols(w1t, w1, 0, 2048)  # w1[e0]
            for nm in loads:
                load_cols(*load_chunks[nm])

            ht_v = ht[:].rearrange("p (k e c) -> p k e c", k=KD, e=EPC)

            def pair(pool):
                # Two PSUM banks: matmuls target one bank each; a single
                # eviction spans both.  Warm matmuls (if any) precede the
                # real groups on the same banks - same engine, in-order.
                pp = pool.tile([128, 2 * 512], f32, name="pp")
                for _ in range(next(wslot)):
                    nc.tensor.matmul(
                        pp[:, 0:512], lhsT=wz[:, 0:128], rhs=wz[:],
                        start=True, stop=True,
                    )
                return pp

            def l1_matmul(e, t, k2, out_ap):
                base = ((e * KS + t) * K2 + k2) * 256
                nc.tensor.matmul(
                    out_ap,
                    lhsT=w1t[:, base : base + 256].rearrange("p (i m) -> p i m", i=2),
                    rhs=ht_v[:, 2 * k2 : 2 * k2 + 2, e],
                    start=(k2 == 0),
                    stop=(k2 == K2 - 1),
                    perf_mode=DR,
                )

            def l2_matmul(e, m, out_ap, hid_v):
                base = (e * KD + m) * 256
                nc.tensor.matmul(
                    out_ap,
                    lhsT=w2t[:, base : base + 256].rearrange("p (i m) -> p i m", i=2),
                    rhs=hid_v,
                    start=True,
                    stop=True,
                    perf_mode=DR,
                )

            ACT, DVE = 0, 1

            def evict(eng, dst_cols, src):
                # out = ps/WSCALE cast to fp8
                if eng == ACT:
                    nc.scalar.activation(dst_cols, src, copy_f, scale=1.0 / WSCALE)
                elif eng == DVE:
                    nc.vector.tensor_scalar_mul(dst_cols, src, 1.0 / WSCALE)
                else:  # Pool / gpsimd as a third eviction engine
                    nc.gpsimd.tensor_scalar_mul(dst_cols, src, 1.0 / WSCALE)

            def evict_pair(eng, dst_cols, pp):
                src = pp[:].rearrange("p (t x) -> p t x", t=2)[:, :, 0:C]
                evict(eng, dst_cols.rearrange("p (t c) -> p t c", t=2), src)

            def store(e, m0, m1, issuer=None):
                # Mid-stream stores ride SP (idle after loads, keeps ACT.SEQ
                # free to dispatch evictions); the final one rides ACT right
                # behind its own last eviction.
                (issuer or nc.sync).dma_start(
                    out=yT[:, (e * KD + m0) * C : (e * KD + m1) * C],
                    in_=ybig[e][:, m0 * C : m1 * C],
                )

            ji = iter(ev)

            def evict_pair(eng, dst_cols, pp):
                src = pp[:].rearrange("p (t x) -> p t x", t=2)[:, :, 0:C]
                evict(eng, dst_cols.rearrange("p (t c) -> p t c", t=2), src)

            def store(e, m0, m1, issuer=None):
                # Mid-stream stores ride the idle SWDGE path (Pool) so the
                # final SP/ACT stores are not queued behind them on a SEQ.
                (issuer or nc.gpsimd).dma_start(
                    out=yT[:, (e * KD + m0) * C : (e * KD + m1) * C],
                    in_=ybig[e][:, m0 * C : m1 * C],
                )

            for e in range(EPC):
                # --- L1: hid[s,c] over 4 DoubleRow chunks of d --------------
                pp1 = pair(ps1_pool)
                for k2 in range(K2):
                    for t in range(KS):
                        l1_matmul(e, t, k2, pp1[:, t * 512 : t * 512 + C])
                evict_pair(next(ji), hid[e][:, :], pp1)

                # --- L2: one DoubleRow matmul per 128-row output tile -------
                hid_v = hid[e][:].rearrange("p (i c) -> p i c", i=KS)
                for mp in range(KD // 2):  # pairs m = 2mp, 2mp+1
                    lo = 2 * mp
                    if e == 1 and mp == 3:
                        # Final pair lands in the (now free) L1 bank pair.
                        pp = pair(ps1_pool)
                        l2_matmul(e, lo, pp[:, 0:C], hid_v)
                        l2_matmul(e, lo + 1, pp[:, 512 : 512 + C], hid_v)
                        evict_pair(next(ji), ybig[e][:, lo * C : (lo + 2) * C], pp)
                    else:
                        pp = pair(ps2_pool)
                        l2_matmul(e, lo, pp[:, 0:C], hid_v)
                        l2_matmul(e, lo + 1, pp[:, 512 : 512 + C], hid_v)
                        evict_pair(next(ji), ybig[e][:, lo * C : (lo + 2) * C], pp)
                    if mp == 1:  # m0-3 ready
                        store(e, 0, 4)
                    elif mp == 2 and e == 1:  # m4-5 ready
                        store(e, 4, 6, issuer=nc.sync)
                    elif mp == 3:
                        if e == 0:
                            store(e, 4, 8)
                        else:
                            store(e, 6, 8, issuer=nc.scalar)

    nc.compile()
    return nc


def kernel(x, y_index, W_in, b_in, W_out, b_out):
    global LAST_RESULT
    from concourse.bass_utils import run_bass_kernel_spmd

    x = np.asarray(x, dtype=np.float32)
    W_in = np.asarray(W_in, dtype=np.float32)
    b_in = np.asarray(b_in, dtype=np.float32)
    W_out = np.asarray(W_out, dtype=np.float32)
    b_out = np.asarray(b_out, dtype=np.float32)
    eidx = np.asarray(y_index).reshape(-1).astype(np.int64)

    counts = np.bincount(eidx, minlength=NB)
    C = max(276, int(-(-counts.max() // 4) * 4))  # capacity per expert

    if C > 512:
        # Extreme expert skew would overflow a PSUM bank (512 f32 free dim);
        # fall back to exact host math rather than ship a broken program.
        out = np.empty_like(x)
        h_full = np.maximum(x, 0.0)
        for e in range(NB):
            m = eidx == e
            if m.any():
                hid = h_full[m] @ W_in[e].T + b_in[e]
                out[m] = x[m] + hid @ W_out[e].T + b_out[e]
        return out

    # --- host dispatch: group tokens by expert ---------------------------
    order = np.argsort(eidx, kind="stable")
    starts = np.zeros(NB + 1, dtype=np.int64)
    np.cumsum(counts, out=starts[1:])

    h = np.maximum(x, 0.0)
    Xg = np.zeros((NB, C, D), dtype=np.float32)
    for e in range(NB):
        toks = order[starts[e] : starts[e + 1]]
        Xg[e, : counts[e]] = h[toks]

    # Fold both biases into one host-side per-expert vector (exact f32).
    bML = np.einsum("eds,es->ed", W_out, b_in) + b_out  # [NB, D]

    # hT: [core, 128, (k, e, c)] - value = h[token (e,c), 128k + p]
    hT_all = np.ascontiguousarray(
        Xg.astype(F8)
        .reshape(NCORES, EPC, C, KD, 128)
        .transpose(0, 4, 3, 1, 2)
        .reshape(NCORES, 128, KD * EPC * C)
    )
    # w1: [core, 128, (e, t, k2, i, m)] = W_in[e, 128t+m, 256k2+128i+p] * 256
    w1_all = np.ascontiguousarray(
        (W_in * WSCALE)
        .astype(F8)
        .reshape(NCORES, EPC, KS, 128, K2, 2, 128)
        .transpose(0, 6, 1, 2, 4, 5, 3)
        .reshape(NCORES, 128, EPC * 2048)
    )
    # w2: [core, 128, (e, m, i, j)] = W_out[e, 128m+j, 128i+p] * 256
    w2_all = np.ascontiguousarray(
        (W_out * WSCALE)
        .astype(F8)
        .reshape(NCORES, EPC, KD, 128, KS, 128)
        .transpose(0, 5, 1, 2, 4, 3)
        .reshape(NCORES, 128, EPC * 2048)
    )

    if C not in _programs:
        _programs[C] = _build_program(C)
    nc = _programs[C]

    in_maps = [
        {"hT": hT_all[i], "w1": w1_all[i], "w2": w2_all[i]} for i in range(NCORES)
    ]

    trace = bool(int(os.environ.get("KERNEL_TRACE", "0")))
    res = run_bass_kernel_spmd(nc, in_maps, list(range(NCORES)), trace=trace)
    LAST_RESULT = res

    # --- host gather: decode fp8, add folded bias, scatter ---------------
    out = np.empty_like(x)
    Yg = np.stack(
        [np.asarray(r["yT"]).astype(np.float32) for r in res.results]
    )  # [NCORES, 128, EPC*KD*C]
    Yg = (
        Yg.reshape(NCORES, 128, EPC, KD, C)
        .transpose(0, 2, 4, 3, 1)
        .reshape(NB, C, D)
    )
    for e in range(NB):
        toks = order[starts[e] : starts[e + 1]]
        out[toks] = x[toks] + Yg[e, : counts[e]] + bML[e]
    return out
